# revision 29
# baseline (speedup 1.0000x reference)
"""MultiHeadCrossAttention kernel for 8 trn2 NeuronCores.

Reference computation (fp32, per batch b):
    q = Q[b] @ W_q.T ; k = K[b] @ W_k.T ; v = V[b] @ W_v.T      (heads on columns)
    per head h: S = (q_h @ k_h.T) / 8 ; E = exp(S); A = E / E.sum(-1)
    out[b] = concat_h(A @ v_h) @ W_o.T ; rows with mask==0 zeroed

Sharding: 8 cores = (batch b in {0,1}) x (head-group hg in {0..3}, 4 heads each).
Each core computes a partial output  out_part[b] = concat(heads hg) @ W_o[:, cols].T
and the host sums the 4 partials per batch (bf16 partials, f32 accumulate).

Single fused pipeline, ScalarE(exp)-paced:
  - The attention j-loop (128-key chunks) is the backbone: per (qp, pair, j)
    one [128,1024] Exp ACTIVATE (~1.07us) paces everything; scores / PV /
    denominator matmuls plus projection and W_o "filler" units are interleaved
    into the PE queue so the whole kernel runs inside the exp stream.
  - Scores: two row-tiled matmuls (K=64, heads of a pair on PE row groups
    0:64 / 64:128) run concurrently.
  - PV: two col-tiled matmuls (M=64 each, col strips 0:64 / 64:128, distinct
    rhs = the two heads' exp columns) run concurrently -> acc[128,512] holds
    both heads' [64 dims, 512 q].
  - Denominator: M=1 col-tiled matmuls (lhsT = ones column) accumulate
    per-head exp row-sums into one PSUM bank (rows 0/32/64/96) across j.
  - Normalize: reciprocal of denominators, DRAM-bounce broadcast across
    partitions, one DVE mul into the W_o lhsT layout (bf16).
  - PSUM: scores 2x[128,1024] (4 banks) + 2 acc + 1 denom + 1 proj = 8.
"""

import numpy as np
import ml_dtypes

import concourse.bass as bass
import concourse.bacc as bacc
import concourse.mybir as mybir
import concourse.tile as tile
from contextlib import ExitStack

F32 = mybir.dt.float32
BF16 = mybir.dt.bfloat16
AF = mybir.ActivationFunctionType

B = 2
SEQ = 2048          # Sq == Sk
D = 1024            # model dim
DL = 256            # local head dims per core (4 heads x 64)
HL = 4              # local heads
DH = 64             # head dim
NCORES = 8

_PROGRAM = None


def build_program(debug=False):
    nc = bacc.Bacc("TRN2", target_bir_lowering=False)

    # inputs are pre-tiled on the host so every DMA is contiguous:
    # x*: [qb, du, p, (a s)]  tile[p, a, s] = x.T[du*512 + a*128 + p, qb*512 + s]
    xqT = nc.declare_dram_parameter("xqT", [4, 2, 128, 2048], BF16, isOutput=False)
    xkT = nc.declare_dram_parameter("xkT", [4, 2, 128, 2048], BF16, isOutput=False)
    xvT = nc.declare_dram_parameter("xvT", [4, 2, 128, 2048], BF16, isOutput=False)
    wq = nc.declare_dram_parameter("wq", [128, 8, DL], BF16, isOutput=False)
    wk = nc.declare_dram_parameter("wk", [128, 8, DL], BF16, isOutput=False)
    wv = nc.declare_dram_parameter("wv", [128, 8, DL], BF16, isOutput=False)
    wo = nc.declare_dram_parameter("wo", [128, 2, D], BF16, isOutput=False)
    # maskq[p, qp, s] = mask[qp*512 + s] for every partition p: lets the mask
    # fold into the reciprocal (1 DVE mul per qp) instead of per-W_o-tile.
    maskq = nc.declare_dram_parameter("maskq", [128, 4, 512], BF16, isOutput=False)
    out_part = nc.declare_dram_parameter("out_part", [SEQ, D], BF16, isOutput=True)

    dbg = {}
    if debug:
        for nm, shp in [("dbg_kT", [128, 2048]), ("dbg_qT", [128, 2048]),
                        ("dbg_vb", [128, 256]), ("dbg_e", [128, 1024]),
                        ("dbg_den", [128, 512]), ("dbg_acc", [128, 512]),
                        ("dbg_rb", [128, 1024]), ("dbg_outT", [128, 1024])]:
            dbg[nm] = nc.declare_dram_parameter(nm, shp, F32, isOutput=True)

    with tile.TileContext(nc) as tc, ExitStack() as ctx:
        const = ctx.enter_context(tc.tile_pool(name="const", bufs=1))
        proj = ctx.enter_context(tc.tile_pool(name="proj", bufs=1))
        epool = ctx.enter_context(tc.tile_pool(name="epool", bufs=6))
        espool = ctx.enter_context(tc.tile_pool(name="espool", bufs=4))
        opool = ctx.enter_context(tc.tile_pool(name="opool", bufs=2))
        ospool = ctx.enter_context(tc.tile_pool(name="ospool", bufs=3))
        rpool = ctx.enter_context(tc.tile_pool(name="rpool", bufs=2))
        apool = ctx.enter_context(tc.tile_pool(name="apool", bufs=4))
        if debug:
            dbgp = ctx.enter_context(tc.tile_pool(name="dbgp", bufs=1))
        stp = ctx.enter_context(tc.tile_pool(name="stp", bufs=2, space="PSUM"))
        accp = ctx.enter_context(tc.tile_pool(name="accp", bufs=2, space="PSUM"))
        denp = ctx.enter_context(tc.tile_pool(name="denp", bufs=1, space="PSUM"))
        pp = ctx.enter_context(tc.tile_pool(name="pp", bufs=1, space="PSUM"))

        # ---------------- constants / persistent SBUF ----------------
        wq_sb = const.tile([128, 8, DL], BF16)
        wk_sb = const.tile([128, 8, DL], BF16)
        wv_sb = const.tile([128, 8, DL], BF16)
        wo_sb = const.tile([128, 2, D], BF16)
        maskq_sb = const.tile([128, 4, 512], BF16)
        ones_sb = const.tile([128, 64], BF16)
        nc.vector.memset(ones_sb[:], 1.0)
        # selectors for the reciprocal broadcast: pair hp's matmul picks rhs
        # row 64*hp (even head -> out rows 0:64) and 64*hp+32 (odd head ->
        # out rows 64:128). Everything stays partition-aligned: the DVE
        # reads inputs at the OUTPUT's partition base, so no op may shift.
        sel_sb = const.tile([128, 2, 128], BF16)
        nc.vector.memset(sel_sb[:], 0.0)
        for hp in range(2):
            nc.vector.memset(sel_sb[64 * hp:64 * hp + 1, hp, 0:64], 1.0)
            nc.vector.memset(sel_sb[64 * hp + 32:64 * hp + 33, hp, 64:128], 1.0)


        # all x input chunks live in SBUF for the whole kernel: x[t][qb][du]
        # tile[p, a, s] = x?T[du*512 + a*128 + p, qb*512 + s]
        xq_t = [[proj.tile([128, 4, 512], BF16, name=f"xq{qb}{du}") for du in range(2)]
                for qb in range(4)]
        xk_t = [[proj.tile([128, 4, 512], BF16, name=f"xk{qb}{du}") for du in range(2)]
                for qb in range(4)]
        xv_t = [[proj.tile([128, 4, 512], BF16, name=f"xv{qb}{du}") for du in range(2)]
                for qb in range(4)]

        kTs = [proj.tile([128, SEQ], BF16, name=f"kT{dm}") for dm in range(2)]
        qTs = [proj.tile([128, SEQ], BF16, name=f"qT{dm}") for dm in range(2)]
        vb = [proj.tile([128, DL], BF16, name=f"vb{j}") for j in range(16)]

        def dma_x(xT, t, qb, du):
            nc.sync.dma_start(
                t[:], xT[qb, du].rearrange("p (a s) -> p a s", a=4))

        # ---------------- startup DMAs (deadline order) ----------------
        # critical path split over two HWDGE queues (sync + scalar), bulk
        # x tiles on the gpsimd SWDGE queue so issue cost stays off both.
        nc.sync.dma_start(wq_sb[:], wq[:])
        nc.scalar.dma_start(wk_sb[:], wk[:])
        dma_x(xqT, xq_t[0][0], 0, 0)
        nc.scalar.dma_start(
            xk_t[0][0][:], xkT[0, 0].rearrange("p (a s) -> p a s", a=4))
        dma_x(xqT, xq_t[0][1], 0, 1)
        nc.scalar.dma_start(
            xk_t[0][1][:], xkT[0, 1].rearrange("p (a s) -> p a s", a=4))
        nc.sync.dma_start(wv_sb[:], wv[:])
        dma_x(xvT, xv_t[0][0], 0, 0)
        dma_x(xvT, xv_t[0][1], 0, 1)
        order = [("k", 1), ("v", 1), ("k", 2), ("v", 2), ("k", 3), ("v", 3),
                 ("q", 1), ("q", 2), ("q", 3)]
        srcs = {"k": (xkT, xk_t), "v": (xvT, xv_t), "q": (xqT, xq_t)}
        for t, qb in order:
            xT, tiles = srcs[t]
            dma_x(xT, tiles[qb][0], qb, 0)
            dma_x(xT, tiles[qb][1], qb, 1)
        nc.sync.dma_start(wo_sb[:], wo[:])
        nc.sync.dma_start(maskq_sb[:], maskq[:])

        # PE warm-up: dummy matmuls on already-initialized SBUF flip the
        # HAM clock gate to 8/8 during the startup DMA wait, so the first
        # projections run at full clock.  A second batch reads wq_sb (first
        # DMA to land) so the PE stays busy right up to the first proj —
        # otherwise a >3.4us idle gap re-throttles the clock to 4/8.
        warm_ps = pp.tile([128, 512], F32, tag="pp", name="warm")
        for _ in range(80):
            nc.tensor.matmul(
                warm_ps[0:64, 0:64],
                lhsT=ones_sb[:, 0:64],
                rhs=ones_sb[:, 0:64],
                start=True,
                stop=True,
            )
        warm_ps2 = pp.tile([128, 512], F32, tag="pp", name="warmb")
        for _ in range(48):
            nc.tensor.matmul(
                warm_ps2[0:64, 0:64],
                lhsT=wq_sb[:, 0, 0:64],
                rhs=wq_sb[:, 0, 0:64],
                start=True,
                stop=True,
            )
        warm_ps3 = pp.tile([128, 512], F32, tag="pp", name="warmc")
        for _ in range(24):
            nc.tensor.matmul(
                warm_ps3[0:64, 0:64],
                lhsT=xq_t[0][0][:, 0, 0:64],
                rhs=xq_t[0][0][:, 0, 0:64],
                start=True,
                stop=True,
            )
        warm_ps4 = pp.tile([128, 512], F32, tag="pp", name="warmd")
        for _ in range(16):
            nc.tensor.matmul(
                warm_ps4[0:64, 0:64],
                lhsT=xk_t[0][0][:, 0, 0:64],
                rhs=xk_t[0][0][:, 0, 0:64],
                start=True,
                stop=True,
            )

        # ---------------- filler units ----------------
        # filler psum ping-pongs between the pp bank and the den bank (den
        # tiles are allocated lazily inside den_mms, so the bank is free for
        # fillers mid-qp).  Without this, every unit's first matmul waits on
        # the previous unit's PSUM->SBUF cast.
        _fp = [0]

        def fill_pool():
            _fp[0] ^= 1
            return (pp, "pp") if _fp[0] else (denp, "den")

        def proj_unit(w_sb, x_qb, dst, dm, qb, pool=None):
            """dst[:, qb*512:...] = (w[:, dm-chunk].T @ x-block), f32->bf16."""
            pl, tag = pool or fill_pool()
            ps = pl.tile([128, 512], F32, tag=tag)
            for ki in range(8):
                nc.tensor.matmul(
                    ps[:],
                    lhsT=w_sb[:, ki, dm * 128:(dm + 1) * 128],
                    rhs=x_qb[ki // 4][:, ki % 4, :],
                    start=(ki == 0),
                    stop=(ki == 7),
                )
            nc.vector.tensor_copy(dst[:, qb * 512:(qb + 1) * 512], ps[:])

        def vb_unit(j):
            """vb[j][128 keys, 256 dl] = xv chunk @ wv."""
            qb, c = j // 4, j % 4
            pl, tag = fill_pool()
            ps = pl.tile([128, 512], F32, tag=tag)
            for ki in range(8):
                nc.tensor.matmul(
                    ps[:, 0:DL],
                    lhsT=xv_t[qb][ki // 4][:, ki % 4, c * 128:(c + 1) * 128],
                    rhs=wv_sb[:, ki, :],
                    start=(ki == 0),
                    stop=(ki == 7),
                )
            nc.vector.tensor_copy(vb[j][:], ps[:, 0:DL])

        outTs = {}
        wo_state = {}

        def wo_mm(qp, mq, oc, pool, tag):
            ps = pool.tile([128, 512], F32, tag=tag,
                           name=f"wops{qp}_{mq}_{oc}")
            for kc in range(2):
                nc.tensor.matmul(
                    ps[:],
                    lhsT=outTs[qp][:, kc, mq * 128:(mq + 1) * 128],
                    rhs=wo_sb[:, kc, oc * 512:(oc + 1) * 512],
                    start=(kc == 0),
                    stop=(kc == 1),
                )
            return ps

        def wo_stage1(qp, mq, pool=None):
            pool = pool or fill_pool()
            o_sb = ospool.tile([128, 1024], BF16, tag="o", name=f"wo{qp}_{mq}")
            wo_state[(qp, mq)] = (o_sb, wo_mm(qp, mq, 0, *pool))

        def wo_stage2(qp, mq, pool=None, scalar_cp=False):
            pool = pool or fill_pool()
            qg = qp * 4 + mq
            o_sb, ps0 = wo_state[(qp, mq)]
            if scalar_cp:
                # tail only: ScalarE is idle after the last exp, so half the
                # PSUM->SBUF copies run there, halving the DVE tail chain.
                nc.scalar.copy(o_sb[:, 0:512], ps0[:])
                nc.sync.dma_start(
                    out_part[qg * 128:(qg + 1) * 128, 0:512], o_sb[:, 0:512])
            else:
                nc.vector.tensor_copy(o_sb[:, 0:512], ps0[:])
            wo_state[(qp, mq)] = (o_sb, wo_mm(qp, mq, 1, *pool))

        def wo_stage3(qp, mq, pool=None, split_dma=False):
            qg = qp * 4 + mq
            o_sb, ps1 = wo_state.pop((qp, mq))
            nc.vector.tensor_copy(o_sb[:, 512:1024], ps1[:])
            if split_dma:
                nc.sync.dma_start(
                    out_part[qg * 128:(qg + 1) * 128, 512:1024],
                    o_sb[:, 512:1024])
            else:
                nc.sync.dma_start(out_part[qg * 128:(qg + 1) * 128, :], o_sb[:])

        norm_state = {}
        qp_res = {}
        dens = {}

        def norm_front(qp, scalar_cp=False):
            acc, den = qp_res[qp], dens[qp]
            acc_sb = [apool.tile([128, 512], F32, tag="accsb",
                                 name=f"accsb{qp}_{i}") for i in range(2)]
            for hp in range(2):
                if scalar_cp:
                    nc.scalar.copy(acc_sb[hp][:], acc[hp][:])
                else:
                    nc.vector.tensor_copy(acc_sb[hp][:], acc[hp][:])
            r_f32 = rpool.tile([128, 512], F32, tag="rf")
            nc.vector.reciprocal_approx_fast(out=r_f32[:], in_=den[:])
            # bf16 cast fused with the row-mask multiply: masked q columns
            # get r=0, so the whole output row zeroes through outT @ W_o.
            r_bf = rpool.tile([128, 512], BF16, tag="rb")
            nc.vector.tensor_mul(r_bf[:], r_f32[:], maskq_sb[:, qp, :])
            norm_state[qp] = (acc_sb, r_bf)

        def norm_back(qp):
            outT_sb = opool.tile([128, 2, 512], BF16, tag="outT",
                                 name=f"outT{qp}")
            outTs[qp] = outT_sb
            acc_sb, r_bf = norm_state.pop(qp)
            for hp in range(2):
                rb_ps = pp.tile([128, 512], F32, tag="pp", name=f"rb{qp}_{hp}")
                nc.tensor.matmul(
                    rb_ps[:],
                    lhsT=sel_sb[0:64 * hp + 33, hp, :],
                    rhs=r_bf[0:64 * hp + 33, :],
                    start=True,
                    stop=True,
                )
                nc.vector.tensor_mul(
                    outT_sb[:, hp, :], acc_sb[hp][:], rb_ps[:]
                )

        # per-(qp) filler schedules: list of callables per j slot
        def make_fillers(qp):
            slots = [[] for _ in range(16)]
            if qp == 0:
                # kT blocks 1..3 (deadline j=4kb), v chunks 2..15 (deadline
                # j+1: PV(j) drains at slot j+1).  Slot 0 carries the startup
                # block (minis + full kT0/qT0 units), so everything else
                # shifts one slot later than its deadline allows.
                slots[1].append(lambda: proj_unit(wk_sb, xk_t[1], kTs[0], 0, 1))
                slots[2].append(lambda: proj_unit(wk_sb, xk_t[1], kTs[1], 1, 1))
                slots[5].append(lambda: proj_unit(wk_sb, xk_t[2], kTs[0], 0, 2))
                slots[6].append(lambda: proj_unit(wk_sb, xk_t[2], kTs[1], 1, 2))
                slots[9].append(lambda: proj_unit(wk_sb, xk_t[3], kTs[0], 0, 3))
                slots[10].append(lambda: proj_unit(wk_sb, xk_t[3], kTs[1], 1, 3))
                for j in range(2, 16):
                    slots[j - 1].append(lambda j=j: vb_unit(j))
                slots[13].append(lambda: proj_unit(wq_sb, xq_t[1], qTs[0], 0, 1))
                slots[14].append(lambda: proj_unit(wq_sb, xq_t[1], qTs[1], 1, 1))
            else:
                slots[0].append(lambda qp=qp: (den_mms(qp - 1),
                                               norm_front(qp - 1)))
                slots[1].append(lambda qp=qp: norm_back(qp - 1))
                for i, mq in enumerate(range(4)):
                    b = 2 + 3 * i
                    slots[b].append(lambda qp=qp, mq=mq: wo_stage1(qp - 1, mq))
                    slots[b + 1].append(lambda qp=qp, mq=mq: wo_stage2(qp - 1, mq))
                    slots[b + 2].append(lambda qp=qp, mq=mq: wo_stage3(qp - 1, mq))
                if qp < 3:
                    slots[6].append(
                        lambda qp=qp: proj_unit(wq_sb, xq_t[qp + 1], qTs[0], 0, qp + 1))
                    slots[9].append(
                        lambda qp=qp: proj_unit(wq_sb, xq_t[qp + 1], qTs[1], 1, qp + 1))
            return slots

        # ---------------- startup projections ----------------
        # (scores/exp for (qp0, j0) are emitted inside the qp loop right
        # after each pair's kT/qT block lands, via the startup flag)

        # ---------------- attention ----------------
        def sc_exp(qp, hp, j):
            """scores + exp for (qp, pair hp, key chunk j) -> e tile.

            One 128x128 LDWEIGHTS covers both row-tiled matmuls (the two
            heads' kT slices stack on the partition dim), so the pair pays
            one ~107ns weight load instead of two serial ones.
            """
            st = stp.tile([128, 1024], F32, tag="st")
            nc.tensor.ldweights(kTs[hp][:, j * 128:(j + 1) * 128])
            for hi in range(2):
                r0 = hi * 64
                mm = nc.tensor.matmul(
                    st[:, hi * 512:(hi + 1) * 512],
                    lhsT=kTs[hp][r0:r0 + 64, j * 128:(j + 1) * 128],
                    rhs=qTs[hp][r0:r0 + 64, qp * 512:(qp + 1) * 512],
                    start=True,
                    stop=True,
                )
                mm.ins.ldweights = False
            e_t = epool.tile([128, 1024], BF16, tag="e")
            nc.scalar.activation(out=e_t[:], in_=st[:], func=AF.Exp, scale=0.125)
            if debug and (qp, hp, j) == (0, 0, 0):
                ecp = dbgp.tile([128, 1024], F32, tag="dbgcp", name="ecp")
                nc.vector.tensor_copy(ecp[:], e_t[:])
                nc.sync.dma_start(dbg["dbg_e"][:], ecp[:])
            return e_t

        esums = {}

        def pv_dn(qp, hp, j, e_t, acc):
            """PV (col-tiled pair) for chunk j + DVE exp-sum accumulation.

            The denominator no longer runs on the PE per chunk: a bf16
            running sum of the exp tiles accumulates on the DVE (2x mode,
            ~594ns per [128,1024] add) and den_mms() reduces it with four
            single matmuls at the end of the qp.  Saves ~38us of PE time.
            """
            nc.tensor.ldweights(vb[j][:, hp * 128:(hp + 1) * 128])
            for hi in range(2):
                mm = nc.tensor.matmul(
                    acc[hi * 64:(hi + 1) * 64, :],
                    lhsT=vb[j][:, hp * 128 + hi * 64: hp * 128 + (hi + 1) * 64],
                    rhs=e_t[:, hi * 512:(hi + 1) * 512],
                    start=(j == 0),
                    stop=(j == 15),
                )
                mm.ins.ldweights = False
            if qp == 3 and j == 15:
                # tail: skip the last DVE add; den_mms(3) folds this tile
                # in directly with an accumulating second matmul, taking
                # ~600ns off the tail critical path.
                e15[hp] = e_t
            elif j == 0:
                es = espool.tile([128, 1024], BF16, tag="esum",
                                 name=f"es{qp}_{hp}")
                esums[(qp, hp)] = es
                nc.vector.tensor_copy(es[:], e_t[:])
            else:
                es = esums[(qp, hp)]
                nc.vector.tensor_add(es[:], es[:], e_t[:])

        e15 = {}

        def den_mms(qp):
            """Column-sum the accumulated exp tiles into the den PSUM rows."""
            den = denp.tile([128, 512], F32, tag="den", name=f"den{qp}")
            dens[qp] = den
            for hp in range(2):
                es = esums.pop((qp, hp))
                tiles = [es]
                if qp == 3:
                    tiles.append(e15.pop(hp))
                for hi in range(2):
                    r = 32 * (2 * hp + hi)
                    for ti, tl in enumerate(tiles):
                        nc.tensor.matmul(
                            den[r:r + 32, :],
                            lhsT=ones_sb[:, 0:32],
                            rhs=tl[:, hi * 512:(hi + 1) * 512],
                            start=(ti == 0),
                            stop=(ti == len(tiles) - 1),
                            tile_position=(0, r),
                        )

        pend = []  # (qp, hp, j, e_t, acc) awaiting PV (crosses qp)
        for qp in range(4):
            fillers = make_fillers(qp)
            acc = [accp.tile([128, 512], F32, tag="acc", name=f"acc{qp}_{hp}")
                   for hp in range(2)]
            qp_res[qp] = acc
            for j in range(16):
                if qp == 0 and j == 0:
                    # startup: minimal proj before each pair's first scores.
                    # The first scores chunk only needs kT keys 0:128, so a
                    # mini kT unit (N=128 matmuls + a 128-wide cast) gets the
                    # first exp going ~1.5us earlier than a full kT block;
                    # the full kT0 units follow right behind for j>=1.
                    def mini_kt(dm):
                        ps = stp.tile([128, 128], F32, tag="st",
                                      name=f"mkt{dm}")
                        for ki in range(8):
                            nc.tensor.matmul(
                                ps[:],
                                lhsT=wk_sb[:, ki, dm * 128:(dm + 1) * 128],
                                rhs=xk_t[0][ki // 4][:, ki % 4, 0:128],
                                start=(ki == 0),
                                stop=(ki == 7),
                            )
                        nc.vector.tensor_copy(kTs[dm][:, 0:128], ps[:])

                    proj_unit(wq_sb, xq_t[0], qTs[0], 0, 0)
                    mini_kt(0)
                    pend.append((0, 0, 0, sc_exp(0, 0, 0), acc))
                    proj_unit(wq_sb, xq_t[0], qTs[1], 1, 0)
                    mini_kt(1)
                    pend.append((0, 1, 0, sc_exp(0, 1, 0), acc))
                    proj_unit(wk_sb, xk_t[0], kTs[0], 0, 0)
                    proj_unit(wk_sb, xk_t[0], kTs[1], 1, 0)
                    vb_unit(0)
                    vb_unit(1)
                else:
                    pend.append((qp, 0, j, sc_exp(qp, 0, j), acc))
                    pend.append((qp, 1, j, sc_exp(qp, 1, j), acc))
                fl = list(fillers[j])
                drain = []
                while len(pend) > 2:
                    drain.append(pend.pop(0))
                if qp == 0:
                    # interleave fillers between the two PV drains so the
                    # proj-psum copy latency hides under attention matmuls
                    for i, (qp_, hp_, j_, e_, a_) in enumerate(drain):
                        pv_dn(qp_, hp_, j_, e_, a_[hp_])
                        if i < len(fl):
                            fl[i]()
                    for f in fl[len(drain):]:
                        f()
                else:
                    # norm_front/back fillers must see ALL drains of the
                    # previous qp first
                    for qp_, hp_, j_, e_, a_ in drain:
                        pv_dn(qp_, hp_, j_, e_, a_[hp_])
                    for f in fl:
                        f()
        # drain the last qp's trailing chunks, then tail normalize + W_o;
        # warm-up matmuls bridge the PE over the reciprocal latency
        while pend:
            qp_, hp_, j_, e_, a_ = pend.pop(0)
            pv_dn(qp_, hp_, j_, e_, a_[hp_])
        den_mms(3)

        # bridge matmuls: keep the PE clock-gate warm across the tail's
        # DVE-only stretches (recip/mask-mul/normalize) so the final W_o
        # matmuls run at 2.4GHz instead of re-throttled 1.2GHz.  They live
        # in the stp pool, free after the last exp.
        def bridge(n, name):
            bp = stp.tile([128, 64], F32, tag="st", name=name)
            for _ in range(n):
                nc.tensor.matmul(
                    bp[0:64, 0:64], lhsT=ones_sb[:, 0:64],
                    rhs=ones_sb[:, 0:64], start=True, stop=True)

        bridge(36, "br0")
        norm_front(3, scalar_cp=True)
        bridge(24, "br1")
        norm_back(3)
        # last qp's W_o (earlier ones ran as fillers): spread over FOUR
        # independent PSUM pools (all free after NORM(3)) so the
        # mm->copy->mm chains of the four units overlap
        pools = {0: (pp, "pp"), 1: (accp, "acc"), 2: (denp, "den"),
                 3: (stp, "st")}
        wo_stage1(3, 0, pool=pools[0])
        wo_stage1(3, 1, pool=pools[1])
        wo_stage2(3, 0, pool=pools[0], scalar_cp=True)
        wo_stage1(3, 2, pool=pools[2])
        wo_stage2(3, 1, pool=pools[1], scalar_cp=True)
        wo_stage3(3, 0, pool=pools[0], split_dma=True)
        wo_stage1(3, 3, pool=pools[3])
        wo_stage2(3, 2, pool=pools[2], scalar_cp=True)
        wo_stage3(3, 1, pool=pools[1], split_dma=True)
        wo_stage2(3, 3, pool=pools[3], scalar_cp=True)
        wo_stage3(3, 2, pool=pools[2], split_dma=True)
        wo_stage3(3, 3, pool=pools[3], split_dma=True)

        if debug:
            kcp = dbgp.tile([128, 2048], F32, tag="dbgcp", name="kcp")
            nc.vector.tensor_copy(kcp[:], kTs[0][:])
            nc.sync.dma_start(dbg["dbg_kT"][:], kcp[:])
            qcp = dbgp.tile([128, 2048], F32, tag="dbgcp", name="qcp")
            nc.vector.tensor_copy(qcp[:], qTs[0][:])
            nc.sync.dma_start(dbg["dbg_qT"][:], qcp[:])
            vcp = dbgp.tile([128, 256], F32, tag="dbgcp", name="vcp")
            nc.vector.tensor_copy(vcp[:], vb[0][:])
            nc.sync.dma_start(dbg["dbg_vb"][:], vcp[:])

    nc.compile()
    return nc


def _get_program():
    global _PROGRAM
    if _PROGRAM is None:
        _PROGRAM = build_program()
    return _PROGRAM


def make_in_maps(Q, K, V, mask, W_q, W_k, W_v, W_o):
    bf = ml_dtypes.bfloat16
    Q, K, V = (np.asarray(a, np.float32) for a in (Q, K, V))
    W_q, W_k, W_v, W_o = (np.asarray(a, np.float32) for a in (W_q, W_k, W_v, W_o))
    mask = np.asarray(mask)
    def tile_x(xT):
        # [du*512 + a*128 + p, qb*512 + s] -> [qb, du, p, (a s)]
        arr = xT.reshape(2, 4, 128, 4, 512).transpose(3, 0, 2, 1, 4)
        return np.ascontiguousarray(arr.reshape(4, 2, 128, 2048)).astype(bf)

    def tile_w(w):
        # [ki*128 + p, dl] -> [p, ki, dl]
        return np.ascontiguousarray(
            w.reshape(8, 128, DL).transpose(1, 0, 2)).astype(bf)

    in_maps = []
    for core in range(NCORES):
        b, hg = core // 4, core % 4
        c0 = hg * DL
        wo_l = W_o[:, c0:c0 + DL].T  # [dl, dout]
        in_maps.append(
            {
                "xqT": tile_x(Q[b].T),
                "xkT": tile_x(K[b].T),
                "xvT": tile_x(V[b].T),
                "wq": tile_w(W_q[c0:c0 + DL, :].T),
                "wk": tile_w(W_k[c0:c0 + DL, :].T),
                "wv": tile_w(W_v[c0:c0 + DL, :].T),
                "wo": np.ascontiguousarray(
                    wo_l.reshape(2, 128, D).transpose(1, 0, 2)).astype(bf),
                "maskq": np.ascontiguousarray(
                    np.broadcast_to(
                        mask[b].reshape(1, 4, 512).astype(bf), (128, 4, 512))
                ),
            }
        )
    return in_maps


def gather(results):
    out = np.zeros((B, SEQ, D), np.float32)
    for core in range(NCORES):
        out[core // 4] += np.asarray(results[core]["out_part"], np.float32)
    return out


def kernel(Q, K, V, mask, W_q, W_k, W_v, W_o):
    from concourse.bass_utils import run_bass_kernel_spmd

    nc = _get_program()
    in_maps = make_in_maps(Q, K, V, mask, W_q, W_k, W_v, W_o)
    res = run_bass_kernel_spmd(nc, in_maps, list(range(NCORES))).results
    return gather(res)



# revision 30
# speedup vs baseline: 1.0674x; 1.0674x over previous
"""MultiHeadCrossAttention kernel for 8 trn2 NeuronCores.

Reference computation (fp32, per batch b):
    q = Q[b] @ W_q.T ; k = K[b] @ W_k.T ; v = V[b] @ W_v.T      (heads on columns)
    per head h: S = (q_h @ k_h.T) / 8 ; E = exp(S); A = E / E.sum(-1)
    out[b] = concat_h(A @ v_h) @ W_o.T ; rows with mask==0 zeroed

Sharding: 8 cores = (batch b in {0,1}) x (head-group hg in {0..3}, 4 heads each).
Each core computes a partial output  out_part[b] = concat(heads hg) @ W_o[:, cols].T
and the host sums the 4 partials per batch (bf16 partials, f32 accumulate).

Single fused pipeline, ScalarE(exp)-paced:
  - The attention j-loop (128-key chunks) is the backbone: per (qp, pair, j)
    one [128,1024] Exp ACTIVATE (~1.07us) paces everything; scores / PV /
    denominator matmuls plus projection and W_o "filler" units are interleaved
    into the PE queue so the whole kernel runs inside the exp stream.
  - Scores: two row-tiled matmuls (K=64, heads of a pair on PE row groups
    0:64 / 64:128) run concurrently.
  - PV: two col-tiled matmuls (M=64 each, col strips 0:64 / 64:128, distinct
    rhs = the two heads' exp columns) run concurrently -> acc[128,512] holds
    both heads' [64 dims, 512 q].
  - Denominator: M=1 col-tiled matmuls (lhsT = ones column) accumulate
    per-head exp row-sums into one PSUM bank (rows 0/32/64/96) across j.
  - Normalize: reciprocal of denominators, DRAM-bounce broadcast across
    partitions, one DVE mul into the W_o lhsT layout (bf16).
  - PSUM: scores 2x[128,1024] (4 banks) + 2 acc + 1 denom + 1 proj = 8.
"""

import numpy as np
import ml_dtypes

import concourse.bass as bass
import concourse.bacc as bacc
import concourse.mybir as mybir
import concourse.tile as tile
from contextlib import ExitStack

F32 = mybir.dt.float32
BF16 = mybir.dt.bfloat16
AF = mybir.ActivationFunctionType

B = 2
SEQ = 2048          # Sq == Sk
D = 1024            # model dim
DL = 256            # local head dims per core (4 heads x 64)
HL = 4              # local heads
DH = 64             # head dim
NCORES = 8

_PROGRAM = None


def build_program(debug=False):
    nc = bacc.Bacc("TRN2", target_bir_lowering=False)

    # inputs are pre-tiled on the host so every DMA is contiguous:
    # x*: [qb, du, p, (a s)]  tile[p, a, s] = x.T[du*512 + a*128 + p, qb*512 + s]
    xqT = nc.declare_dram_parameter("xqT", [4, 2, 128, 2048], BF16, isOutput=False)
    xkT = nc.declare_dram_parameter("xkT", [4, 2, 128, 2048], BF16, isOutput=False)
    xvT = nc.declare_dram_parameter("xvT", [4, 2, 128, 2048], BF16, isOutput=False)
    wq = nc.declare_dram_parameter("wq", [128, 8, DL], BF16, isOutput=False)
    wk = nc.declare_dram_parameter("wk", [128, 8, DL], BF16, isOutput=False)
    wv = nc.declare_dram_parameter("wv", [128, 8, DL], BF16, isOutput=False)
    wo = nc.declare_dram_parameter("wo", [128, 2, D], BF16, isOutput=False)
    # maskq[p, qp, s] = mask[qp*512 + s] for every partition p: lets the mask
    # fold into the reciprocal (1 DVE mul per qp) instead of per-W_o-tile.
    maskq = nc.declare_dram_parameter("maskq", [128, 4, 512], BF16, isOutput=False)
    out_part = nc.declare_dram_parameter("out_part", [SEQ, D], BF16, isOutput=True)

    dbg = {}
    if debug:
        for nm, shp in [("dbg_kT", [128, 2048]), ("dbg_qT", [128, 2048]),
                        ("dbg_vb", [128, 256]), ("dbg_e", [128, 1024]),
                        ("dbg_den", [128, 512]), ("dbg_acc", [128, 512]),
                        ("dbg_rb", [128, 1024]), ("dbg_outT", [128, 1024])]:
            dbg[nm] = nc.declare_dram_parameter(nm, shp, F32, isOutput=True)

    with tile.TileContext(nc) as tc, ExitStack() as ctx:
        const = ctx.enter_context(tc.tile_pool(name="const", bufs=1))
        proj = ctx.enter_context(tc.tile_pool(name="proj", bufs=1))
        epool = ctx.enter_context(tc.tile_pool(name="epool", bufs=6))
        espool = ctx.enter_context(tc.tile_pool(name="espool", bufs=4))
        opool = ctx.enter_context(tc.tile_pool(name="opool", bufs=2))
        ospool = ctx.enter_context(tc.tile_pool(name="ospool", bufs=3))
        rpool = ctx.enter_context(tc.tile_pool(name="rpool", bufs=2))
        apool = ctx.enter_context(tc.tile_pool(name="apool", bufs=4))
        if debug:
            dbgp = ctx.enter_context(tc.tile_pool(name="dbgp", bufs=1))
        stp = ctx.enter_context(tc.tile_pool(name="stp", bufs=2, space="PSUM"))
        accp = ctx.enter_context(tc.tile_pool(name="accp", bufs=2, space="PSUM"))
        denp = ctx.enter_context(tc.tile_pool(name="denp", bufs=1, space="PSUM"))
        pp = ctx.enter_context(tc.tile_pool(name="pp", bufs=1, space="PSUM"))

        # ---------------- constants / persistent SBUF ----------------
        wq_sb = const.tile([128, 8, DL], BF16)
        wk_sb = const.tile([128, 8, DL], BF16)
        wv_sb = const.tile([128, 8, DL], BF16)
        wo_sb = const.tile([128, 2, D], BF16)
        maskq_sb = const.tile([128, 4, 512], BF16)
        ones_sb = const.tile([128, 64], BF16)
        nc.vector.memset(ones_sb[:], 1.0)
        # selectors for the reciprocal broadcast: pair hp's matmul picks rhs
        # row 64*hp (even head -> out rows 0:64) and 64*hp+32 (odd head ->
        # out rows 64:128). Everything stays partition-aligned: the DVE
        # reads inputs at the OUTPUT's partition base, so no op may shift.
        sel_sb = const.tile([128, 2, 128], BF16)
        nc.vector.memset(sel_sb[:], 0.0)
        for hp in range(2):
            nc.vector.memset(sel_sb[64 * hp:64 * hp + 1, hp, 0:64], 1.0)
            nc.vector.memset(sel_sb[64 * hp + 32:64 * hp + 33, hp, 64:128], 1.0)


        # all x input chunks live in SBUF for the whole kernel: x[t][qb][du]
        # tile[p, a, s] = x?T[du*512 + a*128 + p, qb*512 + s]
        xq_t = [[proj.tile([128, 4, 512], BF16, name=f"xq{qb}{du}") for du in range(2)]
                for qb in range(4)]
        xk_t = [[proj.tile([128, 4, 512], BF16, name=f"xk{qb}{du}") for du in range(2)]
                for qb in range(4)]
        xv_t = [[proj.tile([128, 4, 512], BF16, name=f"xv{qb}{du}") for du in range(2)]
                for qb in range(4)]

        kTs = [proj.tile([128, SEQ], BF16, name=f"kT{dm}") for dm in range(2)]
        qTs = [proj.tile([128, SEQ], BF16, name=f"qT{dm}") for dm in range(2)]
        vb = [proj.tile([128, DL], BF16, name=f"vb{j}") for j in range(16)]

        def dma_x(xT, t, qb, du):
            nc.sync.dma_start(
                t[:], xT[qb, du].rearrange("p (a s) -> p a s", a=4))

        # ---------------- startup DMAs (deadline order) ----------------
        # critical path split over two HWDGE queues (sync + scalar), bulk
        # x tiles on the gpsimd SWDGE queue so issue cost stays off both.
        nc.sync.dma_start(wq_sb[:], wq[:])
        nc.scalar.dma_start(wk_sb[:], wk[:])
        dma_x(xqT, xq_t[0][0], 0, 0)
        nc.scalar.dma_start(
            xk_t[0][0][:], xkT[0, 0].rearrange("p (a s) -> p a s", a=4))
        dma_x(xqT, xq_t[0][1], 0, 1)
        nc.scalar.dma_start(
            xk_t[0][1][:], xkT[0, 1].rearrange("p (a s) -> p a s", a=4))
        nc.sync.dma_start(wv_sb[:], wv[:])
        dma_x(xvT, xv_t[0][0], 0, 0)
        dma_x(xvT, xv_t[0][1], 0, 1)
        order = [("k", 1), ("v", 1), ("k", 2), ("v", 2), ("k", 3), ("v", 3),
                 ("q", 1), ("q", 2), ("q", 3)]
        srcs = {"k": (xkT, xk_t), "v": (xvT, xv_t), "q": (xqT, xq_t)}
        for t, qb in order:
            xT, tiles = srcs[t]
            dma_x(xT, tiles[qb][0], qb, 0)
            dma_x(xT, tiles[qb][1], qb, 1)
        nc.sync.dma_start(wo_sb[:], wo[:])
        nc.sync.dma_start(maskq_sb[:], maskq[:])

        # PE warm-up: dummy matmuls on already-initialized SBUF flip the
        # HAM clock gate to 8/8 during the startup DMA wait, so the first
        # projections run at full clock.  A second batch reads wq_sb (first
        # DMA to land) so the PE stays busy right up to the first proj —
        # otherwise a >3.4us idle gap re-throttles the clock to 4/8.
        warm_ps = pp.tile([128, 512], F32, tag="pp", name="warm")
        for _ in range(80):
            nc.tensor.matmul(
                warm_ps[0:64, 0:64],
                lhsT=ones_sb[:, 0:64],
                rhs=ones_sb[:, 0:64],
                start=True,
                stop=True,
            )
        warm_ps2 = pp.tile([128, 512], F32, tag="pp", name="warmb")
        for _ in range(48):
            nc.tensor.matmul(
                warm_ps2[0:64, 0:64],
                lhsT=wq_sb[:, 0, 0:64],
                rhs=wq_sb[:, 0, 0:64],
                start=True,
                stop=True,
            )
        warm_ps3 = pp.tile([128, 512], F32, tag="pp", name="warmc")
        for _ in range(24):
            nc.tensor.matmul(
                warm_ps3[0:64, 0:64],
                lhsT=xq_t[0][0][:, 0, 0:64],
                rhs=xq_t[0][0][:, 0, 0:64],
                start=True,
                stop=True,
            )
        warm_ps4 = pp.tile([128, 512], F32, tag="pp", name="warmd")
        for _ in range(16):
            nc.tensor.matmul(
                warm_ps4[0:64, 0:64],
                lhsT=xk_t[0][0][:, 0, 0:64],
                rhs=xk_t[0][0][:, 0, 0:64],
                start=True,
                stop=True,
            )

        # ---------------- filler units ----------------
        # filler psum ping-pongs between the pp bank and the den bank (den
        # tiles are allocated lazily inside den_mms, so the bank is free for
        # fillers mid-qp).  Without this, every unit's first matmul waits on
        # the previous unit's PSUM->SBUF cast.
        _fp = [0]

        def fill_pool():
            _fp[0] ^= 1
            return (pp, "pp") if _fp[0] else (denp, "den")

        def proj_unit(w_sb, x_qb, dst, dm, qb, pool=None):
            """dst[:, qb*512:...] = (w[:, dm-chunk].T @ x-block), f32->bf16."""
            pl, tag = pool or fill_pool()
            ps = pl.tile([128, 512], F32, tag=tag)
            for ki in range(8):
                nc.tensor.matmul(
                    ps[:],
                    lhsT=w_sb[:, ki, dm * 128:(dm + 1) * 128],
                    rhs=x_qb[ki // 4][:, ki % 4, :],
                    start=(ki == 0),
                    stop=(ki == 7),
                )
            nc.vector.tensor_copy(dst[:, qb * 512:(qb + 1) * 512], ps[:])

        def vb_unit(j):
            """vb[j][128 keys, 256 dl] = xv chunk @ wv."""
            qb, c = j // 4, j % 4
            pl, tag = fill_pool()
            ps = pl.tile([128, 512], F32, tag=tag)
            for ki in range(8):
                nc.tensor.matmul(
                    ps[:, 0:DL],
                    lhsT=xv_t[qb][ki // 4][:, ki % 4, c * 128:(c + 1) * 128],
                    rhs=wv_sb[:, ki, :],
                    start=(ki == 0),
                    stop=(ki == 7),
                )
            nc.vector.tensor_copy(vb[j][:], ps[:, 0:DL])

        outTs = {}
        wo_state = {}

        def wo_mm(qp, mq, oc, pool, tag):
            ps = pool.tile([128, 512], F32, tag=tag,
                           name=f"wops{qp}_{mq}_{oc}")
            for kc in range(2):
                nc.tensor.matmul(
                    ps[:],
                    lhsT=outTs[qp][:, kc, mq * 128:(mq + 1) * 128],
                    rhs=wo_sb[:, kc, oc * 512:(oc + 1) * 512],
                    start=(kc == 0),
                    stop=(kc == 1),
                )
            return ps

        def wo_stage1(qp, mq, pool=None):
            pool = pool or fill_pool()
            o_sb = ospool.tile([128, 1024], BF16, tag="o", name=f"wo{qp}_{mq}")
            wo_state[(qp, mq)] = (o_sb, wo_mm(qp, mq, 0, *pool))

        def wo_stage2(qp, mq, pool=None, scalar_cp=False):
            pool = pool or fill_pool()
            qg = qp * 4 + mq
            o_sb, ps0 = wo_state[(qp, mq)]
            if scalar_cp:
                # tail only: ScalarE is idle after the last exp, so half the
                # PSUM->SBUF copies run there, halving the DVE tail chain.
                nc.scalar.copy(o_sb[:, 0:512], ps0[:])
                nc.sync.dma_start(
                    out_part[qg * 128:(qg + 1) * 128, 0:512], o_sb[:, 0:512])
            else:
                nc.vector.tensor_copy(o_sb[:, 0:512], ps0[:])
            wo_state[(qp, mq)] = (o_sb, wo_mm(qp, mq, 1, *pool))

        def wo_stage3(qp, mq, pool=None, split_dma=False):
            qg = qp * 4 + mq
            o_sb, ps1 = wo_state.pop((qp, mq))
            nc.vector.tensor_copy(o_sb[:, 512:1024], ps1[:])
            if split_dma:
                nc.sync.dma_start(
                    out_part[qg * 128:(qg + 1) * 128, 512:1024],
                    o_sb[:, 512:1024])
            else:
                nc.sync.dma_start(out_part[qg * 128:(qg + 1) * 128, :], o_sb[:])

        norm_state = {}
        qp_res = {}
        dens = {}

        def norm_front(qp, scalar_cp=False):
            acc, den = qp_res[qp], dens[qp]
            acc_sb = [apool.tile([128, 512], F32, tag="accsb",
                                 name=f"accsb{qp}_{i}") for i in range(2)]
            for hp in range(2):
                if scalar_cp:
                    nc.scalar.copy(acc_sb[hp][:], acc[hp][:])
                else:
                    nc.vector.tensor_copy(acc_sb[hp][:], acc[hp][:])
            r_f32 = rpool.tile([128, 512], F32, tag="rf")
            nc.vector.reciprocal_approx_fast(out=r_f32[:], in_=den[:])
            # bf16 cast fused with the row-mask multiply: masked q columns
            # get r=0, so the whole output row zeroes through outT @ W_o.
            r_bf = rpool.tile([128, 512], BF16, tag="rb")
            nc.vector.tensor_mul(r_bf[:], r_f32[:], maskq_sb[:, qp, :])
            norm_state[qp] = (acc_sb, r_bf)

        def norm_back(qp):
            outT_sb = opool.tile([128, 2, 512], BF16, tag="outT",
                                 name=f"outT{qp}")
            outTs[qp] = outT_sb
            acc_sb, r_bf = norm_state.pop(qp)
            for hp in range(2):
                rb_ps = pp.tile([128, 512], F32, tag="pp", name=f"rb{qp}_{hp}")
                nc.tensor.matmul(
                    rb_ps[:],
                    lhsT=sel_sb[0:64 * hp + 33, hp, :],
                    rhs=r_bf[0:64 * hp + 33, :],
                    start=True,
                    stop=True,
                )
                nc.vector.tensor_mul(
                    outT_sb[:, hp, :], acc_sb[hp][:], rb_ps[:]
                )

        # per-(qp) filler schedules: list of callables per j slot
        def make_fillers(qp):
            slots = [[] for _ in range(16)]
            if qp == 0:
                # kT blocks 1..3 (deadline j=4kb), v chunks 2..15 (deadline
                # j+1: PV(j) drains at slot j+1).  Slot 0 carries the startup
                # block (minis + full kT0/qT0 units), so everything else
                # shifts one slot later than its deadline allows.
                slots[1].append(lambda: proj_unit(wk_sb, xk_t[1], kTs[0], 0, 1))
                slots[2].append(lambda: proj_unit(wk_sb, xk_t[1], kTs[1], 1, 1))
                slots[5].append(lambda: proj_unit(wk_sb, xk_t[2], kTs[0], 0, 2))
                slots[6].append(lambda: proj_unit(wk_sb, xk_t[2], kTs[1], 1, 2))
                slots[9].append(lambda: proj_unit(wk_sb, xk_t[3], kTs[0], 0, 3))
                slots[10].append(lambda: proj_unit(wk_sb, xk_t[3], kTs[1], 1, 3))
                for j in range(2, 16):
                    slots[j - 1].append(lambda j=j: vb_unit(j))
                slots[13].append(lambda: proj_unit(wq_sb, xq_t[1], qTs[0], 0, 1))
                slots[14].append(lambda: proj_unit(wq_sb, xq_t[1], qTs[1], 1, 1))
            else:
                slots[0].append(lambda qp=qp: (den_mms(qp - 1),
                                               norm_front(qp - 1)))
                slots[1].append(lambda qp=qp: norm_back(qp - 1))
                for i, mq in enumerate(range(4)):
                    b = 2 + 3 * i
                    slots[b].append(lambda qp=qp, mq=mq: wo_stage1(qp - 1, mq))
                    slots[b + 1].append(lambda qp=qp, mq=mq: wo_stage2(qp - 1, mq))
                    slots[b + 2].append(lambda qp=qp, mq=mq: wo_stage3(qp - 1, mq))
                if qp < 3:
                    slots[6].append(
                        lambda qp=qp: proj_unit(wq_sb, xq_t[qp + 1], qTs[0], 0, qp + 1))
                    slots[9].append(
                        lambda qp=qp: proj_unit(wq_sb, xq_t[qp + 1], qTs[1], 1, qp + 1))
            return slots

        # ---------------- startup projections ----------------
        # (scores/exp for (qp0, j0) are emitted inside the qp loop right
        # after each pair's kT/qT block lands, via the startup flag)

        # ---------------- attention ----------------
        def sc_exp(qp, hp, j):
            """scores + exp for (qp, pair hp, key chunk j) -> e tile.

            One 128x128 LDWEIGHTS covers both row-tiled matmuls (the two
            heads' kT slices stack on the partition dim), so the pair pays
            one ~107ns weight load instead of two serial ones.
            """
            st = stp.tile([128, 1024], F32, tag="st")
            for hi in range(2):
                r0 = hi * 64
                nc.tensor.matmul(
                    st[:, hi * 512:(hi + 1) * 512],
                    lhsT=kTs[hp][r0:r0 + 64, j * 128:(j + 1) * 128],
                    rhs=qTs[hp][r0:r0 + 64, qp * 512:(qp + 1) * 512],
                    start=True,
                    stop=True,
                )
            e_t = epool.tile([128, 1024], BF16, tag="e")
            nc.scalar.activation(out=e_t[:], in_=st[:], func=AF.Exp, scale=0.125)
            if debug and (qp, hp, j) == (0, 0, 0):
                ecp = dbgp.tile([128, 1024], F32, tag="dbgcp", name="ecp")
                nc.vector.tensor_copy(ecp[:], e_t[:])
                nc.sync.dma_start(dbg["dbg_e"][:], ecp[:])
            return e_t

        esums = {}

        def pv_dn(qp, hp, j, e_t, acc):
            """PV (col-tiled pair) for chunk j + DVE exp-sum accumulation.

            The denominator no longer runs on the PE per chunk: a bf16
            running sum of the exp tiles accumulates on the DVE (2x mode,
            ~594ns per [128,1024] add) and den_mms() reduces it with four
            single matmuls at the end of the qp.  Saves ~38us of PE time.
            """
            for hi in range(2):
                nc.tensor.matmul(
                    acc[hi * 64:(hi + 1) * 64, :],
                    lhsT=vb[j][:, hp * 128 + hi * 64: hp * 128 + (hi + 1) * 64],
                    rhs=e_t[:, hi * 512:(hi + 1) * 512],
                    start=(j == 0),
                    stop=(j == 15),
                )
            if qp == 3 and j == 15:
                # tail: skip the last DVE add; den_mms(3) folds this tile
                # in directly with an accumulating second matmul, taking
                # ~600ns off the tail critical path.
                e15[hp] = e_t
            elif j == 0:
                es = espool.tile([128, 1024], BF16, tag="esum",
                                 name=f"es{qp}_{hp}")
                esums[(qp, hp)] = es
                nc.vector.tensor_copy(es[:], e_t[:])
            else:
                es = esums[(qp, hp)]
                nc.vector.tensor_add(es[:], es[:], e_t[:])

        e15 = {}

        def den_mms(qp):
            """Column-sum the accumulated exp tiles into the den PSUM rows."""
            den = denp.tile([128, 512], F32, tag="den", name=f"den{qp}")
            dens[qp] = den
            for hp in range(2):
                es = esums.pop((qp, hp))
                tiles = [es]
                if qp == 3:
                    tiles.append(e15.pop(hp))
                for hi in range(2):
                    r = 32 * (2 * hp + hi)
                    for ti, tl in enumerate(tiles):
                        nc.tensor.matmul(
                            den[r:r + 32, :],
                            lhsT=ones_sb[:, 0:32],
                            rhs=tl[:, hi * 512:(hi + 1) * 512],
                            start=(ti == 0),
                            stop=(ti == len(tiles) - 1),
                            tile_position=(0, r),
                        )

        pend = []  # (qp, hp, j, e_t, acc) awaiting PV (crosses qp)
        for qp in range(4):
            fillers = make_fillers(qp)
            acc = [accp.tile([128, 512], F32, tag="acc", name=f"acc{qp}_{hp}")
                   for hp in range(2)]
            qp_res[qp] = acc
            for j in range(16):
                if qp == 0 and j == 0:
                    # startup: minimal proj before each pair's first scores.
                    # The first scores chunk only needs kT keys 0:128, so a
                    # mini kT unit (N=128 matmuls + a 128-wide cast) gets the
                    # first exp going ~1.5us earlier than a full kT block;
                    # the full kT0 units follow right behind for j>=1.
                    def mini_kt(dm):
                        ps = stp.tile([128, 128], F32, tag="st",
                                      name=f"mkt{dm}")
                        for ki in range(8):
                            nc.tensor.matmul(
                                ps[:],
                                lhsT=wk_sb[:, ki, dm * 128:(dm + 1) * 128],
                                rhs=xk_t[0][ki // 4][:, ki % 4, 0:128],
                                start=(ki == 0),
                                stop=(ki == 7),
                            )
                        nc.vector.tensor_copy(kTs[dm][:, 0:128], ps[:])

                    proj_unit(wq_sb, xq_t[0], qTs[0], 0, 0)
                    mini_kt(0)
                    pend.append((0, 0, 0, sc_exp(0, 0, 0), acc))
                    proj_unit(wq_sb, xq_t[0], qTs[1], 1, 0)
                    mini_kt(1)
                    pend.append((0, 1, 0, sc_exp(0, 1, 0), acc))
                    proj_unit(wk_sb, xk_t[0], kTs[0], 0, 0)
                    proj_unit(wk_sb, xk_t[0], kTs[1], 1, 0)
                    vb_unit(0)
                    vb_unit(1)
                else:
                    pend.append((qp, 0, j, sc_exp(qp, 0, j), acc))
                    pend.append((qp, 1, j, sc_exp(qp, 1, j), acc))
                fl = list(fillers[j])
                drain = []
                while len(pend) > 2:
                    drain.append(pend.pop(0))
                if qp == 0:
                    # interleave fillers between the two PV drains so the
                    # proj-psum copy latency hides under attention matmuls
                    for i, (qp_, hp_, j_, e_, a_) in enumerate(drain):
                        pv_dn(qp_, hp_, j_, e_, a_[hp_])
                        if i < len(fl):
                            fl[i]()
                    for f in fl[len(drain):]:
                        f()
                else:
                    # norm_front/back fillers must see ALL drains of the
                    # previous qp first
                    for qp_, hp_, j_, e_, a_ in drain:
                        pv_dn(qp_, hp_, j_, e_, a_[hp_])
                    for f in fl:
                        f()
        # drain the last qp's trailing chunks, then tail normalize + W_o;
        # warm-up matmuls bridge the PE over the reciprocal latency
        while pend:
            qp_, hp_, j_, e_, a_ = pend.pop(0)
            pv_dn(qp_, hp_, j_, e_, a_[hp_])
        den_mms(3)

        # bridge matmuls: keep the PE clock-gate warm across the tail's
        # DVE-only stretches (recip/mask-mul/normalize) so the final W_o
        # matmuls run at 2.4GHz instead of re-throttled 1.2GHz.  They live
        # in the stp pool, free after the last exp.
        def bridge(n, name):
            bp = stp.tile([128, 64], F32, tag="st", name=name)
            for _ in range(n):
                nc.tensor.matmul(
                    bp[0:64, 0:64], lhsT=ones_sb[:, 0:64],
                    rhs=ones_sb[:, 0:64], start=True, stop=True)

        bridge(36, "br0")
        norm_front(3, scalar_cp=True)
        bridge(24, "br1")
        norm_back(3)
        # last qp's W_o (earlier ones ran as fillers): spread over FOUR
        # independent PSUM pools (all free after NORM(3)) so the
        # mm->copy->mm chains of the four units overlap
        pools = {0: (pp, "pp"), 1: (accp, "acc"), 2: (denp, "den"),
                 3: (stp, "st")}
        wo_stage1(3, 0, pool=pools[0])
        wo_stage1(3, 1, pool=pools[1])
        wo_stage2(3, 0, pool=pools[0], scalar_cp=True)
        wo_stage1(3, 2, pool=pools[2])
        wo_stage2(3, 1, pool=pools[1], scalar_cp=True)
        wo_stage3(3, 0, pool=pools[0], split_dma=True)
        wo_stage1(3, 3, pool=pools[3])
        wo_stage2(3, 2, pool=pools[2], scalar_cp=True)
        wo_stage3(3, 1, pool=pools[1], split_dma=True)
        wo_stage2(3, 3, pool=pools[3], scalar_cp=True)
        wo_stage3(3, 2, pool=pools[2], split_dma=True)
        wo_stage3(3, 3, pool=pools[3], split_dma=True)

        if debug:
            kcp = dbgp.tile([128, 2048], F32, tag="dbgcp", name="kcp")
            nc.vector.tensor_copy(kcp[:], kTs[0][:])
            nc.sync.dma_start(dbg["dbg_kT"][:], kcp[:])
            qcp = dbgp.tile([128, 2048], F32, tag="dbgcp", name="qcp")
            nc.vector.tensor_copy(qcp[:], qTs[0][:])
            nc.sync.dma_start(dbg["dbg_qT"][:], qcp[:])
            vcp = dbgp.tile([128, 256], F32, tag="dbgcp", name="vcp")
            nc.vector.tensor_copy(vcp[:], vb[0][:])
            nc.sync.dma_start(dbg["dbg_vb"][:], vcp[:])

    nc.compile()
    return nc


def _get_program():
    global _PROGRAM
    if _PROGRAM is None:
        _PROGRAM = build_program()
    return _PROGRAM


def make_in_maps(Q, K, V, mask, W_q, W_k, W_v, W_o):
    bf = ml_dtypes.bfloat16
    Q, K, V = (np.asarray(a, np.float32) for a in (Q, K, V))
    W_q, W_k, W_v, W_o = (np.asarray(a, np.float32) for a in (W_q, W_k, W_v, W_o))
    mask = np.asarray(mask)
    def tile_x(xT):
        # [du*512 + a*128 + p, qb*512 + s] -> [qb, du, p, (a s)]
        arr = xT.reshape(2, 4, 128, 4, 512).transpose(3, 0, 2, 1, 4)
        return np.ascontiguousarray(arr.reshape(4, 2, 128, 2048)).astype(bf)

    def tile_w(w):
        # [ki*128 + p, dl] -> [p, ki, dl]
        return np.ascontiguousarray(
            w.reshape(8, 128, DL).transpose(1, 0, 2)).astype(bf)

    in_maps = []
    for core in range(NCORES):
        b, hg = core // 4, core % 4
        c0 = hg * DL
        wo_l = W_o[:, c0:c0 + DL].T  # [dl, dout]
        in_maps.append(
            {
                "xqT": tile_x(Q[b].T),
                "xkT": tile_x(K[b].T),
                "xvT": tile_x(V[b].T),
                "wq": tile_w(W_q[c0:c0 + DL, :].T),
                "wk": tile_w(W_k[c0:c0 + DL, :].T),
                "wv": tile_w(W_v[c0:c0 + DL, :].T),
                "wo": np.ascontiguousarray(
                    wo_l.reshape(2, 128, D).transpose(1, 0, 2)).astype(bf),
                "maskq": np.ascontiguousarray(
                    np.broadcast_to(
                        mask[b].reshape(1, 4, 512).astype(bf), (128, 4, 512))
                ),
            }
        )
    return in_maps


def gather(results):
    out = np.zeros((B, SEQ, D), np.float32)
    for core in range(NCORES):
        out[core // 4] += np.asarray(results[core]["out_part"], np.float32)
    return out


def kernel(Q, K, V, mask, W_q, W_k, W_v, W_o):
    from concourse.bass_utils import run_bass_kernel_spmd

    nc = _get_program()
    in_maps = make_in_maps(Q, K, V, mask, W_q, W_k, W_v, W_o)
    res = run_bass_kernel_spmd(nc, in_maps, list(range(NCORES))).results
    return gather(res)



# revision 35
# speedup vs baseline: 1.0800x; 1.0117x over previous
"""MultiHeadCrossAttention kernel for 8 trn2 NeuronCores.

Reference computation (fp32, per batch b):
    q = Q[b] @ W_q.T ; k = K[b] @ W_k.T ; v = V[b] @ W_v.T      (heads on columns)
    per head h: S = (q_h @ k_h.T) / 8 ; E = exp(S); A = E / E.sum(-1)
    out[b] = concat_h(A @ v_h) @ W_o.T ; rows with mask==0 zeroed

Sharding: 8 cores = (batch b in {0,1}) x (head-group hg in {0..3}, 4 heads each).
Each core computes a partial output  out_part[b] = concat(heads hg) @ W_o[:, cols].T
and the host sums the 4 partials per batch (bf16 partials, f32 accumulate).

Single fused pipeline, jointly paced by the PE and the ScalarE exp stream:
  - The attention j-loop (128-key chunks) is the backbone: per (qp, pair, j)
    one [128,1024] Exp ACTIVATE (~1.0us) paces everything; scores / PV
    matmuls plus projection and W_o "filler" units are interleaved into the
    PE queue so the whole kernel runs inside the exp stream.
  - Scores: two row-tiled matmuls (K=64, heads of a pair on PE row groups
    0:64 / 64:128) run concurrently (~346ns/pair incl. the exposed LDW).
  - PV: two col-tiled matmuls (M=64 each, col strips 0:64 / 64:128, distinct
    rhs = the two heads' exp columns) run concurrently -> acc[128,512] holds
    both heads' [64 dims, 512 q].
  - Denominator: a bf16 running sum of the exp tiles accumulates on the DVE
    (2x mode); at each qp boundary four single matmuls (lhsT = ones column)
    column-sum it into den PSUM rows 0/32/64/96.  This keeps ~38us of
    ones-matmul streaming off the PE, the bottleneck engine.
  - Normalize: reciprocal of denominators, mask folded in via one DVE mul
    (maskq tile), sel-matmul broadcast across partitions, DVE mul into the
    W_o lhsT layout (bf16).
  - Filler PSUM ping-pongs between the pp and den banks (den allocated
    lazily at qp boundaries) so unit N+1's matmuls overlap unit N's cast.
  - Warm-up matmul batches (gated on successive input DMAs) hold the PE
    HAM clock gate at 8/8 through the startup DMA wait; bridge matmuls do
    the same across the tail's DVE-only normalize stretch.
  - Tail W_o copies split between ScalarE (idle after the last exp) and
    DVE; final output DMAs issued in halves as each copy lands.
  - PSUM: scores 2x[128,1024] (4 banks) + 2 acc + 1 den/filler + 1 filler.
"""

import numpy as np
import ml_dtypes

import concourse.bass as bass
import concourse.bacc as bacc
import concourse.mybir as mybir
import concourse.tile as tile
from contextlib import ExitStack

F32 = mybir.dt.float32
BF16 = mybir.dt.bfloat16
AF = mybir.ActivationFunctionType

B = 2
SEQ = 2048          # Sq == Sk
D = 1024            # model dim
DL = 256            # local head dims per core (4 heads x 64)
HL = 4              # local heads
DH = 64             # head dim
NCORES = 8

_PROGRAM = None


def build_program(debug=False):
    nc = bacc.Bacc("TRN2", target_bir_lowering=False)

    # inputs are pre-tiled on the host so every DMA is contiguous:
    # x*: [qb, du, p, (a s)]  tile[p, a, s] = x.T[du*512 + a*128 + p, qb*512 + s]
    xqT = nc.declare_dram_parameter("xqT", [4, 2, 128, 2048], BF16, isOutput=False)
    xkT = nc.declare_dram_parameter("xkT", [4, 2, 128, 2048], BF16, isOutput=False)
    xvT = nc.declare_dram_parameter("xvT", [4, 2, 128, 2048], BF16, isOutput=False)
    wq = nc.declare_dram_parameter("wq", [128, 8, DL], BF16, isOutput=False)
    wk = nc.declare_dram_parameter("wk", [128, 8, DL], BF16, isOutput=False)
    wv = nc.declare_dram_parameter("wv", [128, 8, DL], BF16, isOutput=False)
    wo = nc.declare_dram_parameter("wo", [128, 2, D], BF16, isOutput=False)
    # maskq[p, qp, s] = mask[qp*512 + s] for every partition p: lets the mask
    # fold into the reciprocal (1 DVE mul per qp) instead of per-W_o-tile.
    maskq = nc.declare_dram_parameter("maskq", [128, 4, 512], BF16, isOutput=False)
    out_part = nc.declare_dram_parameter("out_part", [SEQ, D], BF16, isOutput=True)

    dbg = {}
    if debug:
        for nm, shp in [("dbg_kT", [128, 2048]), ("dbg_qT", [128, 2048]),
                        ("dbg_vb", [128, 256]), ("dbg_e", [128, 1024]),
                        ("dbg_den", [128, 512]), ("dbg_acc", [128, 512]),
                        ("dbg_rb", [128, 1024]), ("dbg_outT", [128, 1024])]:
            dbg[nm] = nc.declare_dram_parameter(nm, shp, F32, isOutput=True)

    with tile.TileContext(nc) as tc, ExitStack() as ctx:
        const = ctx.enter_context(tc.tile_pool(name="const", bufs=1))
        proj = ctx.enter_context(tc.tile_pool(name="proj", bufs=1))
        epool = ctx.enter_context(tc.tile_pool(name="epool", bufs=6))
        espool = ctx.enter_context(tc.tile_pool(name="espool", bufs=4))
        opool = ctx.enter_context(tc.tile_pool(name="opool", bufs=2))
        ospool = ctx.enter_context(tc.tile_pool(name="ospool", bufs=3))
        rpool = ctx.enter_context(tc.tile_pool(name="rpool", bufs=2))
        apool = ctx.enter_context(tc.tile_pool(name="apool", bufs=4))
        if debug:
            dbgp = ctx.enter_context(tc.tile_pool(name="dbgp", bufs=1))
        stp = ctx.enter_context(tc.tile_pool(name="stp", bufs=2, space="PSUM"))
        accp = ctx.enter_context(tc.tile_pool(name="accp", bufs=2, space="PSUM"))
        denp = ctx.enter_context(tc.tile_pool(name="denp", bufs=1, space="PSUM"))
        pp = ctx.enter_context(tc.tile_pool(name="pp", bufs=1, space="PSUM"))

        # ---------------- constants / persistent SBUF ----------------
        wq_sb = const.tile([128, 8, DL], BF16)
        wk_sb = const.tile([128, 8, DL], BF16)
        wv_sb = const.tile([128, 8, DL], BF16)
        wo_sb = const.tile([128, 2, D], BF16)
        maskq_sb = const.tile([128, 4, 512], BF16)
        ones_sb = const.tile([128, 64], BF16)
        nc.vector.memset(ones_sb[:], 1.0)
        # selectors for the reciprocal broadcast: pair hp's matmul picks rhs
        # row 64*hp (even head -> out rows 0:64) and 64*hp+32 (odd head ->
        # out rows 64:128). Everything stays partition-aligned: the DVE
        # reads inputs at the OUTPUT's partition base, so no op may shift.
        sel_sb = const.tile([128, 2, 128], BF16)
        nc.vector.memset(sel_sb[:], 0.0)
        for hp in range(2):
            nc.vector.memset(sel_sb[64 * hp:64 * hp + 1, hp, 0:64], 1.0)
            nc.vector.memset(sel_sb[64 * hp + 32:64 * hp + 33, hp, 64:128], 1.0)


        # all x input chunks live in SBUF for the whole kernel: x[t][qb][du]
        # tile[p, a, s] = x?T[du*512 + a*128 + p, qb*512 + s]
        xq_t = [[proj.tile([128, 4, 512], BF16, name=f"xq{qb}{du}") for du in range(2)]
                for qb in range(4)]
        xk_t = [[proj.tile([128, 4, 512], BF16, name=f"xk{qb}{du}") for du in range(2)]
                for qb in range(4)]
        xv_t = [[proj.tile([128, 4, 512], BF16, name=f"xv{qb}{du}") for du in range(2)]
                for qb in range(4)]

        kTs = [proj.tile([128, SEQ], BF16, name=f"kT{dm}") for dm in range(2)]
        qTs = [proj.tile([128, SEQ], BF16, name=f"qT{dm}") for dm in range(2)]
        vb = [proj.tile([128, DL], BF16, name=f"vb{j}") for j in range(16)]

        def dma_x(xT, t, qb, du):
            nc.sync.dma_start(
                t[:], xT[qb, du].rearrange("p (a s) -> p a s", a=4))

        # ---------------- startup DMAs (deadline order) ----------------
        # critical path split over two HWDGE queues (sync + scalar), bulk
        # x tiles on the gpsimd SWDGE queue so issue cost stays off both.
        nc.sync.dma_start(wq_sb[:], wq[:])
        nc.scalar.dma_start(wk_sb[:], wk[:])
        dma_x(xqT, xq_t[0][0], 0, 0)
        nc.scalar.dma_start(
            xk_t[0][0][:], xkT[0, 0].rearrange("p (a s) -> p a s", a=4))
        dma_x(xqT, xq_t[0][1], 0, 1)
        nc.scalar.dma_start(
            xk_t[0][1][:], xkT[0, 1].rearrange("p (a s) -> p a s", a=4))
        nc.sync.dma_start(wv_sb[:], wv[:])
        dma_x(xvT, xv_t[0][0], 0, 0)
        dma_x(xvT, xv_t[0][1], 0, 1)
        order = [("k", 1), ("v", 1), ("k", 2), ("v", 2), ("k", 3), ("v", 3),
                 ("q", 1), ("q", 2), ("q", 3)]
        srcs = {"k": (xkT, xk_t), "v": (xvT, xv_t), "q": (xqT, xq_t)}
        for t, qb in order:
            xT, tiles = srcs[t]
            dma_x(xT, tiles[qb][0], qb, 0)
            dma_x(xT, tiles[qb][1], qb, 1)
        nc.sync.dma_start(wo_sb[:], wo[:])
        nc.sync.dma_start(maskq_sb[:], maskq[:])

        # PE warm-up: dummy matmuls on already-initialized SBUF flip the
        # HAM clock gate to 8/8 during the startup DMA wait, so the first
        # projections run at full clock.  A second batch reads wq_sb (first
        # DMA to land) so the PE stays busy right up to the first proj —
        # otherwise a >3.4us idle gap re-throttles the clock to 4/8.
        warm_ps = pp.tile([128, 512], F32, tag="pp", name="warm")
        for _ in range(80):
            nc.tensor.matmul(
                warm_ps[0:64, 0:64],
                lhsT=ones_sb[:, 0:64],
                rhs=ones_sb[:, 0:64],
                start=True,
                stop=True,
            )
        warm_ps2 = pp.tile([128, 512], F32, tag="pp", name="warmb")
        for _ in range(48):
            nc.tensor.matmul(
                warm_ps2[0:64, 0:64],
                lhsT=wq_sb[:, 0, 0:64],
                rhs=wq_sb[:, 0, 0:64],
                start=True,
                stop=True,
            )
        warm_ps3 = pp.tile([128, 512], F32, tag="pp", name="warmc")
        for _ in range(24):
            nc.tensor.matmul(
                warm_ps3[0:64, 0:64],
                lhsT=xq_t[0][0][:, 0, 0:64],
                rhs=xq_t[0][0][:, 0, 0:64],
                start=True,
                stop=True,
            )
        warm_ps4 = pp.tile([128, 512], F32, tag="pp", name="warmd")
        for _ in range(16):
            nc.tensor.matmul(
                warm_ps4[0:64, 0:64],
                lhsT=xk_t[0][0][:, 0, 0:64],
                rhs=xk_t[0][0][:, 0, 0:64],
                start=True,
                stop=True,
            )

        # ---------------- filler units ----------------
        # filler psum ping-pongs between the pp bank and the den bank (den
        # tiles are allocated lazily inside den_mms, so the bank is free for
        # fillers mid-qp).  Without this, every unit's first matmul waits on
        # the previous unit's PSUM->SBUF cast.
        _fp = [0]

        def fill_pool():
            _fp[0] ^= 1
            return (pp, "pp") if _fp[0] else (denp, "den")

        def proj_unit(w_sb, x_qb, dst, dm, qb, pool=None):
            """dst[:, qb*512:...] = (w[:, dm-chunk].T @ x-block), f32->bf16."""
            pl, tag = pool or fill_pool()
            ps = pl.tile([128, 512], F32, tag=tag)
            for ki in range(8):
                nc.tensor.matmul(
                    ps[:],
                    lhsT=w_sb[:, ki, dm * 128:(dm + 1) * 128],
                    rhs=x_qb[ki // 4][:, ki % 4, :],
                    start=(ki == 0),
                    stop=(ki == 7),
                )
            nc.vector.tensor_copy(dst[:, qb * 512:(qb + 1) * 512], ps[:])

        def vb_unit(j):
            """vb[j][128 keys, 256 dl] = xv chunk @ wv."""
            qb, c = j // 4, j % 4
            pl, tag = fill_pool()
            ps = pl.tile([128, 512], F32, tag=tag)
            for ki in range(8):
                nc.tensor.matmul(
                    ps[:, 0:DL],
                    lhsT=xv_t[qb][ki // 4][:, ki % 4, c * 128:(c + 1) * 128],
                    rhs=wv_sb[:, ki, :],
                    start=(ki == 0),
                    stop=(ki == 7),
                )
            nc.vector.tensor_copy(vb[j][:], ps[:, 0:DL])

        outTs = {}
        wo_state = {}

        def wo_mm(qp, mq, oc, pool, tag):
            ps = pool.tile([128, 512], F32, tag=tag,
                           name=f"wops{qp}_{mq}_{oc}")
            for kc in range(2):
                nc.tensor.matmul(
                    ps[:],
                    lhsT=outTs[qp][:, kc, mq * 128:(mq + 1) * 128],
                    rhs=wo_sb[:, kc, oc * 512:(oc + 1) * 512],
                    start=(kc == 0),
                    stop=(kc == 1),
                )
            return ps

        def wo_stage1(qp, mq, pool=None):
            pool = pool or fill_pool()
            o_sb = ospool.tile([128, 1024], BF16, tag="o", name=f"wo{qp}_{mq}")
            wo_state[(qp, mq)] = (o_sb, wo_mm(qp, mq, 0, *pool))

        def wo_stage2(qp, mq, pool=None, scalar_cp=False):
            pool = pool or fill_pool()
            qg = qp * 4 + mq
            o_sb, ps0 = wo_state[(qp, mq)]
            if scalar_cp:
                # tail only: ScalarE is idle after the last exp, so half the
                # PSUM->SBUF copies run there, halving the DVE tail chain.
                nc.scalar.copy(o_sb[:, 0:512], ps0[:])
                nc.sync.dma_start(
                    out_part[qg * 128:(qg + 1) * 128, 0:512], o_sb[:, 0:512])
            else:
                nc.vector.tensor_copy(o_sb[:, 0:512], ps0[:])
            wo_state[(qp, mq)] = (o_sb, wo_mm(qp, mq, 1, *pool))

        def wo_stage3(qp, mq, pool=None, split_dma=False):
            qg = qp * 4 + mq
            o_sb, ps1 = wo_state.pop((qp, mq))
            nc.vector.tensor_copy(o_sb[:, 512:1024], ps1[:])
            if split_dma:
                nc.sync.dma_start(
                    out_part[qg * 128:(qg + 1) * 128, 512:1024],
                    o_sb[:, 512:1024])
            else:
                nc.sync.dma_start(out_part[qg * 128:(qg + 1) * 128, :], o_sb[:])

        norm_state = {}
        qp_res = {}
        dens = {}

        def norm_front(qp, scalar_cp=False):
            acc, den = qp_res[qp], dens[qp]
            acc_sb = [apool.tile([128, 512], F32, tag="accsb",
                                 name=f"accsb{qp}_{i}") for i in range(2)]
            for hp in range(2):
                if scalar_cp:
                    nc.scalar.copy(acc_sb[hp][:], acc[hp][:])
                else:
                    nc.vector.tensor_copy(acc_sb[hp][:], acc[hp][:])
            r_f32 = rpool.tile([128, 512], F32, tag="rf")
            nc.vector.reciprocal_approx_fast(out=r_f32[:], in_=den[:])
            # bf16 cast fused with the row-mask multiply: masked q columns
            # get r=0, so the whole output row zeroes through outT @ W_o.
            r_bf = rpool.tile([128, 512], BF16, tag="rb")
            nc.vector.tensor_mul(r_bf[:], r_f32[:], maskq_sb[:, qp, :])
            norm_state[qp] = (acc_sb, r_bf)

        def norm_back(qp):
            outT_sb = opool.tile([128, 2, 512], BF16, tag="outT",
                                 name=f"outT{qp}")
            outTs[qp] = outT_sb
            acc_sb, r_bf = norm_state.pop(qp)
            for hp in range(2):
                rb_ps = pp.tile([128, 512], F32, tag="pp", name=f"rb{qp}_{hp}")
                nc.tensor.matmul(
                    rb_ps[:],
                    lhsT=sel_sb[0:64 * hp + 33, hp, :],
                    rhs=r_bf[0:64 * hp + 33, :],
                    start=True,
                    stop=True,
                )
                nc.vector.tensor_mul(
                    outT_sb[:, hp, :], acc_sb[hp][:], rb_ps[:]
                )

        # per-(qp) filler schedules: list of callables per j slot
        def make_fillers(qp):
            slots = [[] for _ in range(16)]
            if qp == 0:
                # kT blocks 1..3 (deadline j=4kb), v chunks 2..15 (deadline
                # j+1: PV(j) drains at slot j+1).  Slot 0 carries the startup
                # block (minis + full kT0/qT0 units), so everything else
                # shifts one slot later than its deadline allows.
                slots[1].append(lambda: proj_unit(wk_sb, xk_t[1], kTs[0], 0, 1))
                slots[2].append(lambda: proj_unit(wk_sb, xk_t[1], kTs[1], 1, 1))
                slots[5].append(lambda: proj_unit(wk_sb, xk_t[2], kTs[0], 0, 2))
                slots[6].append(lambda: proj_unit(wk_sb, xk_t[2], kTs[1], 1, 2))
                slots[9].append(lambda: proj_unit(wk_sb, xk_t[3], kTs[0], 0, 3))
                slots[10].append(lambda: proj_unit(wk_sb, xk_t[3], kTs[1], 1, 3))
                for j in range(2, 16):
                    slots[j - 1].append(lambda j=j: vb_unit(j))
                slots[13].append(lambda: proj_unit(wq_sb, xq_t[1], qTs[0], 0, 1))
                slots[14].append(lambda: proj_unit(wq_sb, xq_t[1], qTs[1], 1, 1))
            else:
                slots[0].append(lambda qp=qp: (den_mms(qp - 1),
                                               norm_front(qp - 1)))
                slots[1].append(lambda qp=qp: norm_back(qp - 1))
                for i, mq in enumerate(range(4)):
                    b = 2 + 3 * i
                    slots[b].append(lambda qp=qp, mq=mq: wo_stage1(qp - 1, mq))
                    slots[b + 1].append(lambda qp=qp, mq=mq: wo_stage2(qp - 1, mq))
                    slots[b + 2].append(lambda qp=qp, mq=mq: wo_stage3(qp - 1, mq))
                if qp < 3:
                    slots[6].append(
                        lambda qp=qp: proj_unit(wq_sb, xq_t[qp + 1], qTs[0], 0, qp + 1))
                    slots[9].append(
                        lambda qp=qp: proj_unit(wq_sb, xq_t[qp + 1], qTs[1], 1, qp + 1))
            return slots

        # ---------------- startup projections ----------------
        # (scores/exp for (qp0, j0) are emitted inside the qp loop right
        # after each pair's kT/qT block lands, via the startup flag)

        # ---------------- attention ----------------
        def sc_exp(qp, hp, j):
            """scores + exp for (qp, pair hp, key chunk j) -> e tile.

            One 128x128 LDWEIGHTS covers both row-tiled matmuls (the two
            heads' kT slices stack on the partition dim), so the pair pays
            one ~107ns weight load instead of two serial ones.
            """
            st = stp.tile([128, 1024], F32, tag="st")
            for hi in range(2):
                r0 = hi * 64
                nc.tensor.matmul(
                    st[:, hi * 512:(hi + 1) * 512],
                    lhsT=kTs[hp][r0:r0 + 64, j * 128:(j + 1) * 128],
                    rhs=qTs[hp][r0:r0 + 64, qp * 512:(qp + 1) * 512],
                    start=True,
                    stop=True,
                )
            e_t = epool.tile([128, 1024], BF16, tag="e")
            nc.scalar.activation(out=e_t[:], in_=st[:], func=AF.Exp, scale=0.125)
            if debug and (qp, hp, j) == (0, 0, 0):
                ecp = dbgp.tile([128, 1024], F32, tag="dbgcp", name="ecp")
                nc.vector.tensor_copy(ecp[:], e_t[:])
                nc.sync.dma_start(dbg["dbg_e"][:], ecp[:])
            return e_t

        esums = {}

        def pv_dn(qp, hp, j, e_t, acc):
            """PV (col-tiled pair) for chunk j + DVE exp-sum accumulation.

            The denominator no longer runs on the PE per chunk: a bf16
            running sum of the exp tiles accumulates on the DVE (2x mode,
            ~594ns per [128,1024] add) and den_mms() reduces it with four
            single matmuls at the end of the qp.  Saves ~38us of PE time.
            """
            for hi in range(2):
                nc.tensor.matmul(
                    acc[hi * 64:(hi + 1) * 64, :],
                    lhsT=vb[j][:, hp * 128 + hi * 64: hp * 128 + (hi + 1) * 64],
                    rhs=e_t[:, hi * 512:(hi + 1) * 512],
                    start=(j == 0),
                    stop=(j == 15),
                )
            if qp == 3 and j == 15:
                # tail: skip the last DVE add; den_mms(3) folds this tile
                # in directly with an accumulating second matmul, taking
                # ~600ns off the tail critical path.
                e15[hp] = e_t
            elif j == 0:
                es = espool.tile([128, 1024], BF16, tag="esum",
                                 name=f"es{qp}_{hp}")
                esums[(qp, hp)] = es
                nc.vector.tensor_copy(es[:], e_t[:])
            else:
                es = esums[(qp, hp)]
                nc.vector.tensor_add(es[:], es[:], e_t[:])

        e15 = {}

        def den_hp(qp, hp, den):
            """Column-sum pair hp's accumulated exp tiles into den rows."""
            es = esums.pop((qp, hp))
            tiles = [es]
            if qp == 3:
                tiles.append(e15.pop(hp))
            for hi in range(2):
                r = 32 * (2 * hp + hi)
                for ti, tl in enumerate(tiles):
                    nc.tensor.matmul(
                        den[r:r + 32, :],
                        lhsT=ones_sb[:, 0:32],
                        rhs=tl[:, hi * 512:(hi + 1) * 512],
                        start=(ti == 0),
                        stop=(ti == len(tiles) - 1),
                        tile_position=(0, r),
                    )

        def den_mms(qp):
            den = denp.tile([128, 512], F32, tag="den", name=f"den{qp}")
            dens[qp] = den
            for hp in range(2):
                den_hp(qp, hp, den)

        pend = []  # (qp, hp, j, e_t, acc) awaiting PV (crosses qp)
        for qp in range(4):
            fillers = make_fillers(qp)
            acc = [accp.tile([128, 512], F32, tag="acc", name=f"acc{qp}_{hp}")
                   for hp in range(2)]
            qp_res[qp] = acc
            for j in range(16):
                if qp == 0 and j == 0:
                    # startup: minimal proj before each pair's first scores.
                    # kT units use the (still-free) scores PSUM pool so their
                    # matmuls overlap the qT unit's PSUM->SBUF cast instead
                    # of serializing on the single pp bank.
                    proj_unit(wq_sb, xq_t[0], qTs[0], 0, 0)
                    proj_unit(wk_sb, xk_t[0], kTs[0], 0, 0, pool=(stp, "st"))
                    pend.append((0, 0, 0, sc_exp(0, 0, 0), acc))
                    proj_unit(wq_sb, xq_t[0], qTs[1], 1, 0)
                    proj_unit(wk_sb, xk_t[0], kTs[1], 1, 0, pool=(stp, "st"))
                    pend.append((0, 1, 0, sc_exp(0, 1, 0), acc))
                    vb_unit(0)
                    vb_unit(1)
                else:
                    pend.append((qp, 0, j, sc_exp(qp, 0, j), acc))
                    pend.append((qp, 1, j, sc_exp(qp, 1, j), acc))
                fl = list(fillers[j])
                drain = []
                while len(pend) > 2:
                    drain.append(pend.pop(0))
                if qp == 0:
                    # interleave fillers between the two PV drains so the
                    # proj-psum copy latency hides under attention matmuls
                    for i, (qp_, hp_, j_, e_, a_) in enumerate(drain):
                        pv_dn(qp_, hp_, j_, e_, a_[hp_])
                        if i < len(fl):
                            fl[i]()
                    for f in fl[len(drain):]:
                        f()
                else:
                    # norm_front/back fillers must see ALL drains of the
                    # previous qp first
                    for qp_, hp_, j_, e_, a_ in drain:
                        pv_dn(qp_, hp_, j_, e_, a_[hp_])
                    for f in fl:
                        f()
        # drain the last qp's trailing chunks, then tail normalize + W_o;
        # warm-up matmuls bridge the PE over the reciprocal latency
        while pend:
            qp_, hp_, j_, e_, a_ = pend.pop(0)
            pv_dn(qp_, hp_, j_, e_, a_[hp_])
        # tail: per-pair normalize pipeline -- hp0's den/recip/broadcast
        # chain runs while the very last exp (pair hp1) is still on the
        # ScalarE.  bridge matmuls keep the PE clock-gate warm across the
        # DVE-only stretches so the W_o matmuls run at full clock.
        def bridge(n, name):
            bp = stp.tile([128, 64], F32, tag="st", name=name)
            for _ in range(n):
                nc.tensor.matmul(
                    bp[0:64, 0:64], lhsT=ones_sb[:, 0:64],
                    rhs=ones_sb[:, 0:64], start=True, stop=True)

        den_mms(3)
        bridge(36, "br0")
        norm_front(3, scalar_cp=True)
        bridge(24, "br1")
        norm_back(3)
        # last qp's W_o (earlier ones ran as fillers): spread over FOUR
        # independent PSUM pools (all free after NORM(3)) so the
        # mm->copy->mm chains of the four units overlap
        pools = {0: (pp, "pp"), 1: (accp, "acc"), 2: (denp, "den"),
                 3: (stp, "st")}
        wo_stage1(3, 0, pool=pools[0])
        wo_stage1(3, 1, pool=pools[1])
        wo_stage2(3, 0, pool=pools[0], scalar_cp=True)
        wo_stage1(3, 2, pool=pools[2])
        wo_stage2(3, 1, pool=pools[1], scalar_cp=True)
        wo_stage3(3, 0, pool=pools[0], split_dma=True)
        wo_stage1(3, 3, pool=pools[3])
        wo_stage2(3, 2, pool=pools[2], scalar_cp=True)
        wo_stage3(3, 1, pool=pools[1], split_dma=True)
        wo_stage2(3, 3, pool=pools[3], scalar_cp=True)
        wo_stage3(3, 2, pool=pools[2], split_dma=True)
        wo_stage3(3, 3, pool=pools[3], split_dma=True)

        if debug:
            kcp = dbgp.tile([128, 2048], F32, tag="dbgcp", name="kcp")
            nc.vector.tensor_copy(kcp[:], kTs[0][:])
            nc.sync.dma_start(dbg["dbg_kT"][:], kcp[:])
            qcp = dbgp.tile([128, 2048], F32, tag="dbgcp", name="qcp")
            nc.vector.tensor_copy(qcp[:], qTs[0][:])
            nc.sync.dma_start(dbg["dbg_qT"][:], qcp[:])
            vcp = dbgp.tile([128, 256], F32, tag="dbgcp", name="vcp")
            nc.vector.tensor_copy(vcp[:], vb[0][:])
            nc.sync.dma_start(dbg["dbg_vb"][:], vcp[:])

    nc.compile()
    return nc


def _get_program():
    global _PROGRAM
    if _PROGRAM is None:
        _PROGRAM = build_program()
    return _PROGRAM


def make_in_maps(Q, K, V, mask, W_q, W_k, W_v, W_o):
    bf = ml_dtypes.bfloat16
    Q, K, V = (np.asarray(a, np.float32) for a in (Q, K, V))
    W_q, W_k, W_v, W_o = (np.asarray(a, np.float32) for a in (W_q, W_k, W_v, W_o))
    mask = np.asarray(mask)
    def tile_x(xT):
        # [du*512 + a*128 + p, qb*512 + s] -> [qb, du, p, (a s)]
        arr = xT.reshape(2, 4, 128, 4, 512).transpose(3, 0, 2, 1, 4)
        return np.ascontiguousarray(arr.reshape(4, 2, 128, 2048)).astype(bf)

    def tile_w(w):
        # [ki*128 + p, dl] -> [p, ki, dl]
        return np.ascontiguousarray(
            w.reshape(8, 128, DL).transpose(1, 0, 2)).astype(bf)

    in_maps = []
    for core in range(NCORES):
        b, hg = core // 4, core % 4
        c0 = hg * DL
        wo_l = W_o[:, c0:c0 + DL].T  # [dl, dout]
        in_maps.append(
            {
                "xqT": tile_x(Q[b].T),
                "xkT": tile_x(K[b].T),
                "xvT": tile_x(V[b].T),
                "wq": tile_w(W_q[c0:c0 + DL, :].T),
                "wk": tile_w(W_k[c0:c0 + DL, :].T),
                "wv": tile_w(W_v[c0:c0 + DL, :].T),
                "wo": np.ascontiguousarray(
                    wo_l.reshape(2, 128, D).transpose(1, 0, 2)).astype(bf),
                "maskq": np.ascontiguousarray(
                    np.broadcast_to(
                        mask[b].reshape(1, 4, 512).astype(bf), (128, 4, 512))
                ),
            }
        )
    return in_maps


def gather(results):
    out = np.zeros((B, SEQ, D), np.float32)
    for core in range(NCORES):
        out[core // 4] += np.asarray(results[core]["out_part"], np.float32)
    return out


def kernel(Q, K, V, mask, W_q, W_k, W_v, W_o):
    from concourse.bass_utils import run_bass_kernel_spmd

    nc = _get_program()
    in_maps = make_in_maps(Q, K, V, mask, W_q, W_k, W_v, W_o)
    res = run_bass_kernel_spmd(nc, in_maps, list(range(NCORES))).results
    return gather(res)

